# revision 2
# baseline (speedup 1.0000x reference)
"""ConvGuidedFilter Trainium2 kernel v2 (8-core SPMD, data parallel over (image, row-half)).

Shapes hardcoded for guide/src [4,3,1024,1024] f32, RADIUS=64, STRIDE=32 (box kernel
of ones). Each core handles one (image b0, row-half hh0) shard: [3, 512, 1024].

Perf design: bf16 on the heavy paths (fp32 matmuls lower to 2 HW passes),
own-image-only upsample, single landing DMA + contiguous pair-sum window trick for
the stats assembly, diagonal (row,col)-tiled MLP with all 4 images concurrent, and
cross-image BN reductions done with block-eye / broadcast matmuls.

Per core:
  P1: cast-load shard to bf16; products g*s (DVE), g*g (ACT); 32-row block sums via
      PE matmuls (lhsT = 1/4096 block indicators) into PSUM [64,1024] pairs; column
      32-segment reduce (DVE) -> [64,32]; column pair-sum -> payload
      PAYF [64 p=(sl,rb), 186 f=(pi,c,j)], written to DRAM in (c,pi,sl,rb,j) order.
  CC: AllGather of the 5952-float payload across 8 cores.
  P2: one landing DMA -> ST [128 p=(b*32+c), 3968 f=(s,hh,rb,j)]; row pair-sum is a
      single strided add (window 31 within 992-blocks); cov/var; L1/L2 1x1-conv MLP
      at tile_position (32b,32b) for the 4 images; BN batch stats via block-eye
      matmul + bn chain + bcast matmul; relu via ACT scale/bias; own-image select
      via per-core input masks (osel matmul for H2A, bmask mul+reduce for means).
  P3 (own image only): A^T via 31 small matmuls; means^T via pair matmuls; col
      upsample (wcol) then row upsample (urow) matmuls; out = mA*guide + mB
      (DVE mul, ACT evac, GPSIMD add); bf16 DMA out, host casts to fp32.
"""

import numpy as np

import concourse.bass as bass
import concourse.bacc as bacc
import concourse.mybir as mybir
import concourse.tile as tile
from concourse.bass_utils import run_bass_kernel_spmd

AF = mybir.ActivationFunctionType
ALU = mybir.AluOpType
AX = mybir.AxisListType
F32 = mybir.dt.float32
BF = mybir.dt.bfloat16

B, C, H, W = 4, 3, 1024, 1024
NCORES = 8
SH = H // 2          # 512 rows per shard
OB = 31              # box output spatial size
RBL = 16             # row blocks per shard (512/32)
PIX = OB * OB        # 961
NPIX = B * PIX       # 3844
EPS = 1e-5
PAY = C * 2 * 2 * RBL * OB   # 5952 floats, order (c, pi, sl, rb, j)


def _build_nc(dump=False):
    nc = bacc.Bacc("TRN2", target_bir_lowering=False, debug=False, num_devices=NCORES)

    g_d = nc.dram_tensor("g_sh", [C, SH, W], F32, kind="ExternalInput")
    s_d = nc.dram_tensor("s_sh", [C, SH, W], F32, kind="ExternalInput")
    bsum_d = nc.dram_tensor("bsum", [128, 64], BF, kind="ExternalInput")
    pairm_d = nc.dram_tensor("pairm", [32, OB], BF, kind="ExternalInput")
    wcol_d = nc.dram_tensor("wcol", [OB, W], BF, kind="ExternalInput")
    urow_d = nc.dram_tensor("urow", [64, SH], BF, kind="ExternalInput")
    w1c_d = nc.dram_tensor("w1c", [128, 32], BF, kind="ExternalInput")
    w1v_d = nc.dram_tensor("w1v", [128, 32], BF, kind="ExternalInput")
    w2r_d = nc.dram_tensor("w2r", [128, 32], BF, kind="ExternalInput")
    w3r_d = nc.dram_tensor("w3r", [128, 4], BF, kind="ExternalInput")
    beye_d = nc.dram_tensor("beye", [128, 32], F32, kind="ExternalInput")
    bcast_d = nc.dram_tensor("bcast", [32, 128], F32, kind="ExternalInput")
    osel_d = nc.dram_tensor("osel", [128, 32], BF, kind="ExternalInput")
    bmask_d = nc.dram_tensor("bmask", [32, B * 2 * C * OB], F32,
                             kind="ExternalInput")
    bn_d = nc.dram_tensor("bn", [32, 5], F32, kind="ExternalInput")
    out_d = nc.dram_tensor("out_sh", [C, SH, W], BF, kind="ExternalOutput")
    dmp = {}
    if dump:
        for nm, shape, dt in [
            ("pay", [64, 186], F32), ("st", [128, 3968], F32),
            ("ps", [128, 3844], F32), ("cv", [128, 1922], BF),
            ("h1a", [128, 961], BF), ("h2a", [128, 961], BF),
            ("sf2", [32, 186], BF), ("ajs", [31, 93], BF),
            ("mxs", [31, 93], BF), ("mys", [31, 93], BF),
            ("bjs", [31, 93], BF), ("wsa", [31, 3072], BF),
            ("wsb", [31, 3072], BF), ("ss1", [128, 2], F32),
            ("ss2", [128, 2], F32), ("h2ob", [32, 961], BF),
        ]:
            dmp[nm] = nc.dram_tensor("d_" + nm, shape, dt, kind="ExternalOutput")

    with tile.TileContext(nc) as tc:
        with (
            tc.tile_pool(name="consts", bufs=1) as consts,
            tc.tile_pool(name="gres", bufs=1) as gres,
            tc.tile_pool(name="persist", bufs=1) as persist,
            tc.tile_pool(name="dram", bufs=1, space="DRAM") as dram,
        ):
            def const(name, dram_t, shape, dt=BF):
                t = consts.tile(shape, dt, tag=name, name=name + "_sb")
                nc.sync.dma_start(t[:], dram_t[:])
                return t

            bsum = const("bsum", bsum_d, [128, 64])
            pairm = const("pairm", pairm_d, [32, OB])
            wcol = const("wcol", wcol_d, [OB, W])
            urow = const("urow", urow_d, [64, SH])
            w1c = const("w1c", w1c_d, [128, 32])
            w1v = const("w1v", w1v_d, [128, 32])
            w2r = const("w2r", w2r_d, [128, 32])
            w3r = const("w3r", w3r_d, [128, 4])
            beye = const("beye", beye_d, [128, 32], F32)
            bcast = const("bcast", bcast_d, [32, 128], F32)
            osel = const("osel", osel_d, [128, 32])
            bmask = const("bmask", bmask_d, [32, B * 2 * C * OB], F32)
            bn = const("bn", bn_d, [32, 5], F32)

            # bf16 resident shard copies: [128, (c,t)*1024]
            G = gres.tile([128, C * 4 * 1024], BF)
            S = gres.tile([128, C * 4 * 1024], BF)

            PAYF = persist.tile([64, 2 * C * OB], F32)
            cc_in = dram.tile([PAY], F32)
            cc_out = dram.tile([NCORES, PAY], F32)

            # ---------------- Phase 1 ----------------
            # Stats order s=(pi,sl): 0=g(mean_x), 1=src(mean_y), 2=g*s, 3=g*g.
            with (
                tc.tile_pool(name="prod", bufs=3) as prod,
                tc.tile_pool(name="psum1", bufs=2, space="PSUM") as psum1,
                tc.tile_pool(name="cred", bufs=2) as cred,
            ):
                for c in range(C):
                    for t in range(4):
                        for dst, src_t in ((G, g_d), (S, s_d)):
                            nc.gpsimd.dma_start(
                                dst[:, (c * 4 + t) * 1024:
                                       (c * 4 + t + 1) * 1024],
                                src_t[c, t * 128:(t + 1) * 128, :],
                            )
                    P = [psum1.tile([64, 1024], F32, tag=f"p1_{pi}",
                                    name=f"P{c}_{pi}") for pi in range(2)]
                    for t in range(4):
                        gsl = G[:, (c * 4 + t) * 1024:(c * 4 + t + 1) * 1024]
                        ssl = S[:, (c * 4 + t) * 1024:(c * 4 + t + 1) * 1024]
                        for s_idx, srct in enumerate((gsl, ssl)):
                            pi, sl = s_idx // 2, s_idx % 2
                            for hh in range(2):
                                nc.tensor.matmul(
                                    P[pi][32 * sl:32 * sl + 16,
                                          hh * 512:hh * 512 + 512],
                                    bsum[:, 16 * t:16 * t + 16],
                                    srct[:, hh * 512:hh * 512 + 512],
                                    start=(t == 0), stop=(t == 3),
                                )
                        gs = prod.tile([128, 1024], BF, tag="gs")
                        nc.vector.tensor_mul(gs[:], gsl, ssl)
                        gg = prod.tile([128, 1024], BF, tag="gg")
                        nc.scalar.activation(gg[:], gsl, AF.Square)
                        for s_idx, srct in enumerate((None, None, gs[:], gg[:])):
                            if srct is None:
                                continue
                            pi, sl = s_idx // 2, s_idx % 2
                            for hh in range(2):
                                nc.tensor.matmul(
                                    P[pi][32 * sl:32 * sl + 16,
                                          hh * 512:hh * 512 + 512],
                                    bsum[:, 16 * t:16 * t + 16],
                                    srct[:, hh * 512:hh * 512 + 512],
                                    start=(t == 0), stop=(t == 3),
                                )
                    for pi in range(2):
                        CR = cred.tile([64, 32], F32, tag=f"cr_{pi}")
                        nc.vector.tensor_reduce(
                            CR[:], P[pi][:].rearrange("p (a b) -> p a b", b=32),
                            axis=AX.X, op=ALU.add,
                        )
                        nc.vector.tensor_add(
                            PAYF[:, pi * 3 * OB + c * OB:pi * 3 * OB + (c + 1) * OB],
                            CR[:, 0:OB], CR[:, 1:32])
                # payload write, target order (c, pi, sl, rb, j)
                ccv = cc_in[:].rearrange("(c p s r j) -> p s r c j",
                                         c=3, p=2, s=2, r=16, j=OB)
                for pi in range(2):
                    for sl in range(2):
                        nc.sync.dma_start(
                            ccv[pi, sl],
                            PAYF[32 * sl:32 * sl + 16,
                                 pi * 93:(pi + 1) * 93].rearrange(
                                "r (c j) -> r c j", c=3))

            if dump:
                nc.sync.dma_start(dmp["pay"][:], PAYF[:])
            nc.gpsimd.collective_compute(
                "AllGather",
                ALU.bypass,
                replica_groups=[list(range(NCORES))],
                ins=[cc_in.opt()],
                outs=[cc_out.opt()],
            )

            # PE keep-warm filler: chained dummy matmuls spanning the
            # collective gap so the HAM stays at K=8/8 into P2/P3.
            with tc.tile_pool(name="psumW", bufs=1, space="PSUM") as psumW:
                WRM = psumW.tile([16, 512], F32, tag="wrm")
                for _ in range(90):
                    nc.tensor.matmul(WRM[:], bsum[:, 0:16], G[:, 0:512],
                                     start=True, stop=True)

            # ---------------- Phase 2 ----------------
            with (
                tc.tile_pool(name="p2", bufs=1) as p2,
                tc.tile_pool(name="small", bufs=2) as small,
            ):
                with tc.tile_pool(name="psumH", bufs=1, space="PSUM") as psumH:
                    # landing: ST [128 p=(b*32+c), 3968 f=(pi,sl,hh,rb,j)]
                    ST = p2.tile([128, 3968], F32)
                    stl = ST[:].rearrange("(b u) (p s h r j) -> b h u p s r j",
                                          b=4, p=2, s=2, h=2, r=16)
                    for b in range(B):
                        for hh in range(2):
                            eng = nc.sync if hh == 0 else nc.gpsimd
                            eng.dma_start(
                                stl[b, hh, 0:3],
                                cc_out[2 * b + hh].rearrange(
                                    "(c p s r j) -> c p s r j",
                                    c=3, p=2, s=2, r=16))

                    # post-landing PE keep-warm chain (covers landing->L1 gap)
                    STb = p2.tile([128, 512], BF, tag="stb")
                    nc.vector.tensor_copy(STb[:], ST[:, 0:512])
                    with tc.tile_pool(name="psumW2", bufs=1,
                                      space="PSUM") as psumW2:
                        WRM2 = psumW2.tile([16, 512], F32, tag="wrm2")
                        for _ in range(56):
                            nc.tensor.matmul(WRM2[:], bsum[:, 0:16], STb[:],
                                             start=True, stop=True)

                    # SF2all: [32 p=(hh,rb), (si,b,c,j)] f32, means stats (pi=0)
                    SF2a = p2.tile([32, B * 2 * C * OB], F32)
                    srcv = cc_out[:].rearrange(
                        "(b h) (c p s r j) -> h p s r b c j",
                        h=2, c=3, p=2, s=2, r=16)
                    sfv = SF2a[:].rearrange("(h r) (s b c j) -> h s r b c j",
                                            h=2, b=B, s=2, c=3)
                    for hh in range(2):
                        for si in range(2):
                            for b in range(B):
                                nc.sync.dma_start(sfv[hh, si, :, b],
                                                   srcv[hh, 0, si, :, b])
                    # own-image select: mask-mul then reduce over b
                    SF2m = p2.tile([32, B * 2 * C * OB], F32)
                    nc.gpsimd.tensor_mul(SF2m[:], SF2a[:], bmask[:])
                    SF2h = p2.tile([32, 2 * 2 * C * OB], F32, tag="sf2h")
                    v4 = SF2m[:].rearrange("p (s b f) -> p s b f", s=2, b=B)
                    nc.gpsimd.tensor_add(
                        SF2h[:].rearrange("p (s b f) -> p s b f", s=2, b=2),
                        v4[:, :, 0:2], v4[:, :, 2:4])
                    SF2 = p2.tile([32, 2 * C * OB], BF)
                    v2 = SF2h[:].rearrange("p (s b f) -> p s b f", s=2, b=2)
                    nc.gpsimd.tensor_add(
                        SF2[:].rearrange("p (s f) -> p s f", s=2),
                        v2[:, :, 0], v2[:, :, 1])
                    if dump:
                        nc.sync.dma_start(dmp["st"][:], ST[:])
                    # row pair-sum (windows of 31 inside 992-stride blocks)
                    PS01 = p2.tile([128, 2 * PIX], F32)
                    PS23 = p2.tile([128, 2 * PIX], F32)
                    stv = ST[:].rearrange("p (s f) -> p s f", s=4)
                    nc.vector.tensor_add(
                        PS01[:].rearrange("p (s f) -> p s f", s=2),
                        stv[:, 0:2, 0:PIX], stv[:, 0:2, OB:OB + PIX])
                    nc.vector.tensor_add(
                        PS23[:].rearrange("p (s f) -> p s f", s=2),
                        stv[:, 2:4, 0:PIX], stv[:, 2:4, OB:OB + PIX])
                    # cov/var -> CV bf16 [128, 2*961]
                    CV = p2.tile([128, 2 * PIX], BF)
                    TM = p2.tile([128, PIX], F32, tag="tm")
                    TM2 = p2.tile([128, PIX], F32, tag="tm2")
                    nc.scalar.activation(TM2[:], PS01[:, 0:PIX], AF.Square)
                    nc.vector.tensor_mul(TM[:], PS01[:, 0:PIX],
                                         PS01[:, PIX:2 * PIX])
                    nc.vector.tensor_sub(CV[:, 0:PIX], PS23[:, 0:PIX], TM[:])
                    nc.gpsimd.tensor_sub(CV[:, PIX:2 * PIX],
                                         PS23[:, PIX:2 * PIX], TM2[:])

                    if dump:
                        nc.sync.dma_start(dmp["ps"][:, 0:2 * PIX], PS01[:])
                        nc.sync.dma_start(dmp["ps"][:, 2 * PIX:4 * PIX],
                                          PS23[:])
                        nc.sync.dma_start(dmp["cv"][:], CV[:])

                    chunks = [(0, 512), (512, PIX)]

                    def mlp_layer(rhs_fn, lhsTs, name):
                        HP = psumH.tile([128, PIX], F32, tag="hp",
                                        name=f"HP_{name}", bufs=2)
                        for b in range(4):
                            for (o, e) in chunks:
                                for li, lt in enumerate(lhsTs):
                                    nc.tensor.matmul(
                                        HP[32 * b:32 * b + 32, o:e],
                                        lt[0][32 * b:32 * b + lt[1], :],
                                        rhs_fn(b, li, o, e),
                                        start=(li == 0),
                                        stop=(li == len(lhsTs) - 1),
                                        tile_position=(32 * b, 32 * b),
                                    )
                        return HP

                    def bn_apply(HP, gcol, bcol, name):
                        SQ = small.tile([128, 2], F32, tag=f"sq_{name}")
                        nc.vector.tensor_reduce(SQ[:, 0:1], HP[:], axis=AX.X,
                                                op=ALU.add)
                        SQs = p2.tile([128, PIX], BF, tag="sqs", bufs=2)
                        nc.scalar.activation(SQs[:], HP[:], AF.Square,
                                             accum_out=SQ[:, 1:2])
                        BNp = psumH.tile([32, 2], F32, tag="bnp")
                        nc.tensor.matmul(BNp[:], beye[:], SQ[:],
                                         start=True, stop=True)
                        MV = small.tile([32, 2], F32, tag=f"mv_{name}")
                        nc.vector.tensor_scalar_mul(MV[:], BNp[:], 1.0 / NPIX)
                        VV = small.tile([32, 1], F32, tag=f"vv_{name}")
                        # VV = m*m - q  (variance negated; Sqrt uses scale=-1)
                        nc.vector.scalar_tensor_tensor(
                            VV[:], MV[:, 0:1], MV[:, 0:1], MV[:, 1:2],
                            op0=ALU.mult, op1=ALU.subtract)
                        SD = small.tile([32, 1], F32, tag=f"sd_{name}")
                        nc.scalar.activation(SD[:], VV[:], AF.Sqrt,
                                             bias=bn[:, 4:5], scale=-1.0)
                        RS = small.tile([32, 1], F32, tag=f"rs_{name}")
                        nc.vector.reciprocal(RS[:], SD[:])
                        SB = small.tile([32, 2], F32, tag=f"sb_{name}")
                        nc.vector.tensor_mul(SB[:, 0:1], RS[:],
                                             bn[:, gcol:gcol + 1])
                        TT = small.tile([32, 1], F32, tag=f"tt_{name}")
                        nc.vector.tensor_mul(TT[:], MV[:, 0:1], SB[:, 0:1])
                        nc.vector.tensor_sub(SB[:, 1:2], bn[:, bcol:bcol + 1],
                                             TT[:])
                        SSp = psumH.tile([128, 2], F32, tag="ssp")
                        nc.tensor.matmul(SSp[:], bcast[:], SB[:],
                                         start=True, stop=True)
                        SS = small.tile([128, 2], F32, tag=f"ss_{name}")
                        nc.scalar.copy(SS[:], SSp[:])
                        if dump:
                            nc.sync.dma_start(
                                dmp["ss1" if name == "l1" else "ss2"][:], SS[:])
                        HA = p2.tile([128, PIX], BF, tag=f"ha_{name}",
                                     name=f"HA_{name}")
                        nc.scalar.activation(HA[:], HP[:], AF.Relu,
                                             scale=SS[:, 0:1], bias=SS[:, 1:2])
                        return HA

                    H1P = mlp_layer(
                        lambda b, li, o, e: CV[32 * b:32 * b + 3,
                                               li * PIX + o:li * PIX + e],
                        [(w1c, 3), (w1v, 3)], "l1")
                    H1A = bn_apply(H1P, 0, 1, "l1")
                    if dump:
                        nc.sync.dma_start(dmp["sf2"][:], SF2[:])
                    H2P = mlp_layer(
                        lambda b, li, o, e: H1A[32 * b:32 * b + 32, o:e],
                        [(w2r, 32)], "l2")
                    H2A = bn_apply(H2P, 2, 3, "l2")
                    if dump:
                        nc.sync.dma_start(dmp["h1a"][:], H1A[:])
                        nc.sync.dma_start(dmp["h2a"][:], H2A[:])

                    # own-image H2A block -> partitions 0..31 (block-select matmul)
                    H2o = psumH.tile([32, PIX], F32, tag="hp", name="H2o",
                                     bufs=2)
                    for (o, e) in chunks:
                        nc.tensor.matmul(H2o[:, o:e], osel[:], H2A[:, o:e],
                                         start=True, stop=True)
                    H2ob = p2.tile([32, PIX], BF, tag="h2ob")
                    nc.scalar.copy(H2ob[:], H2o[:])

                    # AJ [31 j, (i,c)]
                    AJp = psumH.tile([OB, C * OB], F32, tag="cj", bufs=1,
                                     name="AJp")
                    for i in range(OB):
                        nc.tensor.matmul(AJp[:, C * i:C * i + C],
                                         H2ob[:, OB * i:OB * i + OB],
                                         w3r[0:32, 0:C], start=True, stop=True)
                    AJs = p2.tile([OB, C * OB], BF, tag="ajs")
                    nc.scalar.copy(AJs[:], AJp[:])

                    # means^T [31 j, (i,c)] via pair matmuls from SF2
                    MXYp = psumH.tile([OB, 2 * C * OB], F32, tag="cj", bufs=1,
                                      name="MXYp")
                    for si in range(2):
                        for c in range(C):
                            nc.tensor.matmul(
                                MXYp[:, si * C * OB:(si + 1) * C * OB].rearrange(
                                    "p (i c) -> p c i", c=C)[:, c, :],
                                SF2[:, (si * C + c) * OB:(si * C + c + 1) * OB],
                                pairm[:], start=True, stop=True)
                    MXs = p2.tile([OB, C * OB], BF, tag="mxs")
                    nc.scalar.copy(MXs[:], MXYp[:, 0:C * OB])
                    MYs = p2.tile([OB, C * OB], BF, tag="mys")
                    nc.scalar.copy(MYs[:], MXYp[:, C * OB:2 * C * OB])
                    TJ = p2.tile([OB, C * OB], BF, tag="tj")
                    nc.vector.tensor_mul(TJ[:], AJs[:], MXs[:])
                    BJs = p2.tile([OB, C * OB], BF, tag="bjs")
                    nc.vector.tensor_sub(BJs[:], MYs[:], TJ[:])
                    if dump:
                        nc.sync.dma_start(dmp["h2ob"][:], H2ob[:])
                        nc.sync.dma_start(dmp["ajs"][:], AJs[:])
                        nc.sync.dma_start(dmp["mxs"][:], MXs[:])
                        nc.sync.dma_start(dmp["mys"][:], MYs[:])
                        nc.sync.dma_start(dmp["bjs"][:], BJs[:])

                # ---------------- Phase 3 ----------------
                with (
                    tc.tile_pool(name="wide", bufs=1) as wide,
                    tc.tile_pool(name="p3", bufs=3) as p3,
                ):
                    # combined Ws: A at partitions 0..30, B at 32..62
                    WsT = wide.tile([64, C * W], BF, tag="wst", name="WsT")
                    with tc.tile_pool(name="psumU", bufs=2,
                                      space="PSUM") as psumU:
                        for c in range(C):
                            ja = AJs[:].rearrange("p (i c) -> p c i",
                                                  c=C)[:, c, :]
                            jb = BJs[:].rearrange("p (i c) -> p c i",
                                                  c=C)[:, c, :]
                            for hw in range(2):
                                Wp = psumU.tile([64, 512], F32, tag="wp")
                                nc.tensor.matmul(
                                    Wp[0:OB, :], ja,
                                    wcol[:, hw * 512:hw * 512 + 512],
                                    start=True, stop=True,
                                    tile_position=(0, 0))
                                nc.tensor.matmul(
                                    Wp[32:32 + OB, :], jb,
                                    wcol[:, hw * 512:hw * 512 + 512],
                                    start=True, stop=True,
                                    tile_position=(0, 32))
                                nc.scalar.copy(
                                    WsT[:, c * W + hw * 512:
                                        c * W + hw * 512 + 512],
                                    Wp[:])
                    if dump:
                        nc.sync.dma_start(dmp["wsa"][:], WsT[0:OB, :])
                        nc.sync.dma_start(dmp["wsb"][:], WsT[32:32 + OB, :])

                    with tc.tile_pool(name="psumM", bufs=2,
                                      space="PSUM") as psumM:
                        for c in range(C):
                            for rc in range(4):
                                mAp = psumM.tile([128, 1024], F32, tag="map")
                                mBp = psumM.tile([128, 1024], F32, tag="mbp")
                                for hw in range(2):
                                    nc.tensor.matmul(
                                        mAp[:, hw * 512:hw * 512 + 512],
                                        urow[0:OB, rc * 128:(rc + 1) * 128],
                                        WsT[0:OB, c * W + hw * 512:
                                            c * W + hw * 512 + 512],
                                        start=True, stop=True,
                                        tile_position=(0, 0))
                                    nc.tensor.matmul(
                                        mBp[:, hw * 512:hw * 512 + 512],
                                        urow[32:32 + OB,
                                             rc * 128:(rc + 1) * 128],
                                        WsT[32:32 + OB, c * W + hw * 512:
                                            c * W + hw * 512 + 512],
                                        start=True, stop=True,
                                        tile_position=(32, 0))
                                PRD = p3.tile([128, 1024], BF, tag="prd")
                                nc.vector.tensor_mul(
                                    PRD[:], mAp[:],
                                    G[:, (c * 4 + rc) * 1024:
                                         (c * 4 + rc + 1) * 1024])
                                MBS = p3.tile([128, 1024], BF, tag="mbs")
                                nc.scalar.copy(MBS[:], mBp[:])
                                OUT = p3.tile([128, 1024], BF, tag="out")
                                aeng = nc.gpsimd if (c * 4 + rc) % 2 else nc.vector
                                aeng.tensor_add(OUT[:], PRD[:], MBS[:])
                                nc.sync.dma_start(
                                    out_d[c, rc * 128:(rc + 1) * 128, :],
                                    OUT[:])

    nc.compile()
    return nc


_NC_CACHE = {}


def _host_consts():
    import ml_dtypes
    bf = ml_dtypes.bfloat16
    bsum = np.zeros((128, 64), np.float32)
    for t in range(4):
        for k in range(128):
            bsum[k, 16 * t + 4 * t + k // 32] = 1.0 / 4096.0
    pairm = np.zeros((32, OB), np.float32)
    for i in range(OB):
        pairm[i, i] = 1.0
        pairm[i + 1, i] = 1.0

    def interp(n_out):
        xs = np.linspace(0.0, float(OB - 1), n_out).astype(np.float32)
        x0 = np.floor(xs).astype(np.int32)
        x1 = np.minimum(x0 + 1, OB - 1)
        wx = (xs - x0.astype(np.float32)).astype(np.float32)
        M = np.zeros((OB, n_out), np.float32)
        for j in range(n_out):
            M[x0[j], j] += 1.0 - wx[j]
            M[x1[j], j] += wx[j]
        return M

    beye = np.zeros((128, 32), np.float32)
    bcast = np.zeros((32, 128), np.float32)
    for p in range(128):
        beye[p, p % 32] = 1.0
        bcast[p % 32, p] = 1.0
    return (bsum.astype(bf), pairm.astype(bf), interp(W).astype(bf),
            interp(H), beye, bcast, bf)


def _urow64(urow_full, hh0, bf):
    u = np.zeros((64, SH), np.float32)
    u[0:OB] = urow_full[:, SH * hh0:SH * (hh0 + 1)]
    u[32:32 + OB] = urow_full[:, SH * hh0:SH * (hh0 + 1)]
    return u.astype(bf)


def kernel(**inputs):
    guide = np.ascontiguousarray(np.asarray(inputs["guide"], dtype=np.float32))
    src = np.ascontiguousarray(np.asarray(inputs["src"], dtype=np.float32))
    w1 = np.asarray(inputs["w1"], dtype=np.float32)
    w2 = np.asarray(inputs["w2"], dtype=np.float32)
    w3 = np.asarray(inputs["w3"], dtype=np.float32)
    g1 = np.asarray(inputs["g1"], dtype=np.float32)
    b1 = np.asarray(inputs["b1"], dtype=np.float32)
    g2 = np.asarray(inputs["g2"], dtype=np.float32)
    b2 = np.asarray(inputs["b2"], dtype=np.float32)

    import os
    dump = bool(os.environ.get("KV2_DUMP"))
    key = ("nc", dump)
    if key not in _NC_CACHE:
        _NC_CACHE[key] = _build_nc(dump)
    nc = _NC_CACHE[key]

    bsum, pairm, wcol, urow_full, beye, bcast, bf = _host_consts()
    w1c = np.zeros((128, 32), np.float32)
    w1v = np.zeros((128, 32), np.float32)
    w2r = np.zeros((128, 32), np.float32)
    for b in range(4):
        w1c[32 * b:32 * b + 3] = w1[:, 0:3].T
        w1v[32 * b:32 * b + 3] = w1[:, 3:6].T
        w2r[32 * b:32 * b + 32] = w2.T
    w3r = np.zeros((128, 4), np.float32)
    w3r[0:32, 0:3] = w3.T
    bn = np.stack([g1, b1, g2, b2, np.full(32, EPS, np.float32)],
                  axis=1).astype(np.float32)

    in_maps = []
    for k in range(NCORES):
        b0, hh0 = k // 2, k % 2
        osel = np.zeros((128, 32), np.float32)
        for o in range(32):
            osel[32 * b0 + o, o] = 1.0
        bmask = np.zeros((32, 2, B, C * OB), np.float32)
        bmask[:, :, b0, :] = 1.0
        in_maps.append(dict(
            g_sh=np.ascontiguousarray(guide[b0, :, SH * hh0:SH * (hh0 + 1), :]),
            s_sh=np.ascontiguousarray(src[b0, :, SH * hh0:SH * (hh0 + 1), :]),
            bsum=bsum, pairm=pairm, wcol=wcol,
            urow=_urow64(urow_full, hh0, bf),
            w1c=w1c.astype(bf), w1v=w1v.astype(bf), w2r=w2r.astype(bf),
            w3r=w3r.astype(bf), beye=beye, bcast=bcast, bn=bn,
            osel=osel.astype(bf),
            bmask=bmask.reshape(32, -1),
        ))

    res = run_bass_kernel_spmd(nc, in_maps, list(range(NCORES)))
    if dump:
        kernel.dumps = [
            {k: np.asarray(v) for k, v in res.results[kk].items()}
            for kk in range(NCORES)]
    out = np.empty((B, C, H, W), np.float32)
    for k in range(NCORES):
        b0, hh0 = k // 2, k % 2
        out[b0, :, SH * hh0:SH * (hh0 + 1), :] = \
            res.results[k]["out_sh"].astype(np.float32)
    return out
